# revision 1
# baseline (speedup 1.0000x reference)
"""Trainium2 Bass kernel for nn_JResCOPAttn (B=1, L=1024, D=128).

Reference computation:
    a   = x @ Wl.T + bl                        # [L, D]
    tm  = (a[:,None,:] * a[None,:,:]) @ Wlo.T + blo    # [L, L, D]  (never materialized!)
    tm *= (mask != 0)
    tx  = x @ Wl2.T + bl2                      # [L, D]
    y   = x + einsum('cad,ad->cd', tm, tx)
    out = LayerNorm(y) * gamma + beta

Algebraic restructuring used here (per output row c):
    y1[c,d] = sum_e act[c,e] * WloT[e,d] * S_c[e,d]  +  blo[d] * Z[c,d]
    S_c[e,d] = sum_a (mask[c,a]*act[a,e]) * tx[a,d]      (8 accumulating matmuls)
    Z[c,d]   = sum_a mask[c,a] * tx[a,d]                 (one batch of matmuls)
This avoids materializing the 536MB tm tensor entirely.

Sharding: rows c are split across the 8 NeuronCores (128 rows each); x is
replicated so each core computes act/tx for all 1024 source rows locally.
"""

import os
import sys

for _p in ("/opt/trn_rl_repo", "/root/.axon_site/_ro/trn_rl_repo"):
    if os.path.isdir(_p) and _p not in sys.path:
        sys.path.insert(0, _p)

import numpy as np

import concourse.bass as bass
import concourse.tile as tile
from concourse import bacc, mybir
from concourse.bass_utils import run_bass_kernel_spmd
from concourse.masks import make_identity

B, L, D = 1, 1024, 128
NCORES = 8
CB = L // NCORES          # c-rows per core = 128
T = L // 128              # a-tiles = 8
EPS = 1e-5
FP = mybir.dt.float32

# how many of the 8 per-c mask-apply ops run on DVE (rest on ScalarE/ACT)
N_DVE_MASK = 5
QUAD = 4                  # c's sharing one PSUM bank / one G multiply


def build_nc():
    nc = bacc.Bacc("TRN2", target_bir_lowering=False)

    # ---- I/O ----
    xT   = nc.dram_tensor("xT",   [128, L], FP, kind="ExternalInput")    # x^T (feature-major)
    xTb  = nc.dram_tensor("xTb",  [128, CB], FP, kind="ExternalInput")   # this core's block of xT cols
    xrow = nc.dram_tensor("xrow", [CB, D], FP, kind="ExternalInput")     # this core's x rows (residual)
    mT   = nc.dram_tensor("mT",   [128, T, CB], FP, kind="ExternalInput")  # mT[p,t,c] = mask[c0+c, t*128+p]
    WlT  = nc.dram_tensor("WlT",  [128, 128], FP, kind="ExternalInput")  # Wl.T
    Wl2T = nc.dram_tensor("Wl2T", [128, 128], FP, kind="ExternalInput")  # Wl2.T
    Wlo4 = nc.dram_tensor("Wlo4", [128, QUAD, 128], FP, kind="ExternalInput")  # Wlo.T replicated QUADx
    bl   = nc.dram_tensor("bl",   [128, 1], FP, kind="ExternalInput")
    bl2  = nc.dram_tensor("bl2",  [128, 1], FP, kind="ExternalInput")
    blo  = nc.dram_tensor("blo",  [128, 1], FP, kind="ExternalInput")
    gam  = nc.dram_tensor("gam",  [CB, D], FP, kind="ExternalInput")     # gamma broadcast to rows
    bet  = nc.dram_tensor("bet",  [CB, D], FP, kind="ExternalInput")
    out  = nc.dram_tensor("out",  [CB, D], FP, kind="ExternalOutput")

    Ident = mybir.ActivationFunctionType.Identity
    Sqrt = mybir.ActivationFunctionType.Sqrt
    mult = mybir.AluOpType.mult

    with tile.TileContext(nc) as tc:
        with (
            tc.tile_pool(name="singles", bufs=1) as singles,
            tc.tile_pool(name="trps", bufs=2, space="PSUM") as trps,
            tc.tile_pool(name="setps", bufs=2, space="PSUM") as setps,
            tc.tile_pool(name="ma", bufs=3) as ma_pool,
            tc.tile_pool(name="g", bufs=2) as g_pool,
            tc.tile_pool(name="s4", bufs=2, space="PSUM") as s4_pool,
            tc.tile_pool(name="y1tp", bufs=1, space="PSUM") as y1t_pool,
        ):
            # ---- load constants / inputs ----
            sb_xT = singles.tile([128, L], FP)
            nc.sync.dma_start(sb_xT, xT[:, :])
            sb_xTb = singles.tile([128, CB], FP)
            nc.sync.dma_start(sb_xTb, xTb[:, :])
            sb_xrow = singles.tile([CB, D], FP)
            nc.sync.dma_start(sb_xrow, xrow[:, :])
            sb_mT = singles.tile([128, T, CB], FP)
            nc.sync.dma_start(sb_mT, mT[:, :, :])
            sb_WlT = singles.tile([128, 128], FP)
            nc.sync.dma_start(sb_WlT, WlT[:, :])
            sb_Wl2T = singles.tile([128, 128], FP)
            nc.sync.dma_start(sb_Wl2T, Wl2T[:, :])
            sb_Wlo4 = singles.tile([128, QUAD, 128], FP)
            nc.sync.dma_start(sb_Wlo4, Wlo4[:, :, :])
            sb_bl = singles.tile([128, 1], FP)
            nc.sync.dma_start(sb_bl, bl[:, :])
            sb_bl2 = singles.tile([128, 1], FP)
            nc.sync.dma_start(sb_bl2, bl2[:, :])
            sb_blo = singles.tile([128, 1], FP)
            nc.sync.dma_start(sb_blo, blo[:, :])
            sb_gam = singles.tile([CB, D], FP)
            nc.sync.dma_start(sb_gam, gam[:, :])
            sb_bet = singles.tile([CB, D], FP)
            nc.sync.dma_start(sb_bet, bet[:, :])

            ident = singles.tile([128, 128], FP)
            make_identity(nc, ident)
            sb_eps = singles.tile([CB, 1], FP)
            nc.vector.memset(sb_eps, EPS)

            # ---- actT / txT = W @ xT + bias  (feature-major activations) ----
            actT = singles.tile([128, L], FP)
            txT = singles.tile([128, L], FP)
            for h in range(2):
                sl = slice(h * 512, (h + 1) * 512)
                ps_a = setps.tile([128, 512], FP, tag="set_mm")
                nc.tensor.matmul(ps_a, sb_WlT, sb_xT[:, sl], start=True, stop=True)
                nc.scalar.activation(actT[:, sl], ps_a, Ident, bias=sb_bl, scale=1.0)
                ps_t = setps.tile([128, 512], FP, tag="set_mm")
                nc.tensor.matmul(ps_t, sb_Wl2T, sb_xT[:, sl], start=True, stop=True)
                nc.scalar.activation(txT[:, sl], ps_t, Ident, bias=sb_bl2, scale=1.0)

            # actT restricted to this core's c-block (for the reduce matmuls)
            actTb = singles.tile([128, CB], FP)
            ps_b = setps.tile([128, 512], FP, tag="set_mm")
            nc.tensor.matmul(ps_b[:, :CB], sb_WlT, sb_xTb, start=True, stop=True)
            nc.scalar.activation(actTb, ps_b[:, :CB], Ident, bias=sb_bl, scale=1.0)

            # ---- natural-layout act / tx tiles via PE transpose ----
            act_nat = singles.tile([128, T, 128], FP)
            tx_nat = singles.tile([128, T, 128], FP)
            for t in range(T):
                sl = slice(t * 128, (t + 1) * 128)
                p1 = trps.tile([128, 128], FP, tag="tr")
                nc.tensor.transpose(p1, actT[:, sl], ident)
                nc.vector.tensor_copy(act_nat[:, t, :], p1)
                p2 = trps.tile([128, 128], FP, tag="tr")
                nc.tensor.transpose(p2, txT[:, sl], ident)
                nc.vector.tensor_copy(tx_nat[:, t, :], p2)

            # ---- ZT[d,c] = sum_a tx[a,d] * mask[c,a];  bloZT = blo * ZT ----
            zt_ps = setps.tile([128, 512], FP, tag="set_mm")
            for t in range(T):
                nc.tensor.matmul(
                    zt_ps[:, :CB], tx_nat[:, t, :], sb_mT[:, t, :],
                    start=(t == 0), stop=(t == T - 1),
                )
            bloZT = singles.tile([128, CB], FP)
            nc.vector.tensor_scalar_mul(bloZT, zt_ps[:, :CB], sb_blo)

            # ---- main loop over this core's 128 output rows ----
            y1t_ps = y1t_pool.tile([128, CB], FP)  # Y1^T columns, [d, c]
            for cq in range(CB // QUAD):
                s4 = s4_pool.tile([128, QUAD, 128], FP)
                for j in range(QUAD):
                    c = cq * QUAD + j
                    ma = ma_pool.tile([128, T, 128], FP, tag="ma")
                    for t in range(T):
                        if t < N_DVE_MASK:
                            nc.vector.tensor_scalar_mul(
                                ma[:, t, :], act_nat[:, t, :], sb_mT[:, t, c:c + 1]
                            )
                        else:
                            nc.scalar.mul(
                                ma[:, t, :], act_nat[:, t, :], sb_mT[:, t, c:c + 1]
                            )
                    for t in range(T):
                        nc.tensor.matmul(
                            s4[:, j, :], ma[:, t, :], tx_nat[:, t, :],
                            start=(t == 0), stop=(t == T - 1),
                        )
                g4 = g_pool.tile([128, QUAD, 128], FP, tag="g4")
                nc.vector.tensor_mul(g4, s4, sb_Wlo4)
                for j in range(QUAD):
                    c = cq * QUAD + j
                    nc.tensor.matmul(
                        y1t_ps[:, c:c + 1], g4[:, j, :], actTb[:, c:c + 1],
                        start=True, stop=True,
                    )

            # ---- combine, transpose back, residual, LayerNorm ----
            yt_sb = singles.tile([128, CB], FP)
            nc.vector.tensor_add(yt_sb, y1t_ps, bloZT)           # [d, c]
            y_ps = trps.tile([128, 128], FP, tag="tr")
            nc.tensor.transpose(y_ps, yt_sb, ident)              # [c, d]
            y_sb = singles.tile([CB, D], FP)
            nc.vector.tensor_add(y_sb, y_ps, sb_xrow)            # + x residual

            stats = singles.tile([CB, nc.vector.BN_STATS_DIM], FP)
            nc.vector.bn_stats(stats, y_sb)
            mv = singles.tile([CB, 2], FP)
            nc.vector.bn_aggr(mv, stats)
            nc.vector.tensor_scalar_sub(y_sb, y_sb, mv[:, 0:1])  # y - mean
            sd = singles.tile([CB, 1], FP)
            nc.scalar.activation(sd, mv[:, 1:2], Sqrt, bias=sb_eps, scale=1.0)
            rstd = singles.tile([CB, 1], FP)
            nc.vector.reciprocal(rstd, sd)
            nc.vector.tensor_scalar_mul(y_sb, y_sb, rstd)
            nc.vector.tensor_mul(y_sb, y_sb, sb_gam)
            nc.vector.tensor_add(y_sb, y_sb, sb_bet)

            nc.sync.dma_start(out[:, :], y_sb)

    return nc


_NC_CACHE = None


def _get_nc():
    global _NC_CACHE
    if _NC_CACHE is None:
        _NC_CACHE = build_nc()
        _NC_CACHE.finalize()
    return _NC_CACHE


def _prepare_in_maps(x, mask, Wl, bl, Wlo, blo, Wl2, bl2, gamma, beta):
    f32 = np.float32
    x0 = np.ascontiguousarray(np.asarray(x, f32)[0])          # [L, D]
    m = np.asarray(mask)[0].astype(f32)                       # [L, L] (c, a)
    xT = np.ascontiguousarray(x0.T)                           # [128, L]
    WlT = np.ascontiguousarray(np.asarray(Wl, f32).T)
    Wl2T = np.ascontiguousarray(np.asarray(Wl2, f32).T)
    WloT = np.ascontiguousarray(np.asarray(Wlo, f32).T)       # [e, d]
    Wlo4 = np.ascontiguousarray(
        np.broadcast_to(WloT[:, None, :], (128, QUAD, 128)).astype(f32)
    )
    bl_c = np.asarray(bl, f32).reshape(128, 1)
    bl2_c = np.asarray(bl2, f32).reshape(128, 1)
    blo_c = np.asarray(blo, f32).reshape(128, 1)
    gam_b = np.ascontiguousarray(np.broadcast_to(np.asarray(gamma, f32), (CB, D)))
    bet_b = np.ascontiguousarray(np.broadcast_to(np.asarray(beta, f32), (CB, D)))

    in_maps = []
    for k in range(NCORES):
        blk = slice(k * CB, (k + 1) * CB)
        mTk = m[blk, :].T.reshape(T, 128, CB).transpose(1, 0, 2)  # [p, t, c]
        in_maps.append({
            "xT": xT,
            "xTb": np.ascontiguousarray(xT[:, blk]),
            "xrow": np.ascontiguousarray(x0[blk]),
            "mT": np.ascontiguousarray(mTk),
            "WlT": WlT,
            "Wl2T": Wl2T,
            "Wlo4": Wlo4,
            "bl": bl_c,
            "bl2": bl2_c,
            "blo": blo_c,
            "gam": gam_b,
            "bet": bet_b,
        })
    return in_maps


def kernel(x, mask, Wl, bl, Wlo, blo, Wl2, bl2, gamma, beta):
    in_maps = _prepare_in_maps(x, mask, Wl, bl, Wlo, blo, Wl2, bl2, gamma, beta)
    res = run_bass_kernel_spmd(_get_nc(), in_maps, core_ids=list(range(NCORES)))
    y = np.concatenate([res.results[k]["out"] for k in range(NCORES)], axis=0)
    return y.reshape(B, L, D).astype(np.float32)



# revision 12
# speedup vs baseline: 1.8161x; 1.8161x over previous
"""Trainium2 Bass kernel for nn_JResCOPAttn (B=1, L=1024, D=128).

Reference computation:
    act = x @ Wl.T + bl                               # [L, E]  (E = D = 128)
    tm  = (act[:,None,:] * act[None,:,:]) @ Wlo.T + blo   # [L, L, D] (never materialized)
    tm *= (mask != 0)
    tx  = x @ Wl2.T + bl2                             # [L, D]
    y   = x + einsum('cad,ad->cd', tm, tx)
    out = LayerNorm(y) * gamma + beta

Algebraic restructuring (per output row c, channel d):
    y1[c,d] = sum_e act[c,e] * WloT[e,d] * S[c,e,d]  +  blo[d] * Z[c,d]
    S[c,e,d] = sum_a mask[c,a] * act[a,e] * tx[a,d]
    Z[c,d]   = sum_a mask[c,a] * tx[a,d]

Sharding: the e-dimension (128) is split across the 8 cores (16 e's each).
Each core computes P2[a,e,d] = act[a,e]*tx[a,d]*WloT[e,d] for its e-shard,
then S2 = maskT.T @ P2 as one large bf16 matmul (contraction over a=1024,
N=512 streams -> full PE rate; fp32 matmuls are 4x slower on TRN2).
The per-core partials y1p[c,d] = sum_{e in shard} act[c,e]*S2[c,e,d] are
summed with a ReduceScatter so core k ends up owning rows [128k, 128k+128),
where it adds the Z-term + residual and applies LayerNorm.
"""

import os
import sys

for _p in ("/opt/trn_rl_repo", "/root/.axon_site/_ro/trn_rl_repo"):
    if os.path.isdir(_p) and _p not in sys.path:
        sys.path.insert(0, _p)

import numpy as np
import ml_dtypes

import concourse.bass as bass
import concourse.tile as tile
from concourse import bacc, mybir
from concourse.bass_utils import run_bass_kernel_spmd

B, L, D = 1, 1024, 128
NCORES = 8
ESH = 16                  # e-channels per core
T = L // 128              # a-tiles = 8
CB = L // NCORES          # c-rows owned per core after ReduceScatter = 128
EPS = 1e-5
FP = mybir.dt.float32
BF = mybir.dt.bfloat16
BF_NP = ml_dtypes.bfloat16

N_DVE_J = 12              # P2-build: j < N_DVE_J on DVE, rest on gpsimd


def build_nc():
    nc = bacc.Bacc("TRN2", target_bir_lowering=False, num_devices=NCORES)

    # ---- I/O (per-core) ----
    xT    = nc.dram_tensor("xT",    [128, L], BF, kind="ExternalInput")        # x^T (d-major)
    maskT = nc.dram_tensor("maskT", [128, T, L], BF, kind="ExternalInput")     # [p,t,c] = mask[c, 128t+p]
    maskz = nc.dram_tensor("maskz", [128, T, CB], BF, kind="ExternalInput")    # own-shard columns
    WlTk  = nc.dram_tensor("WlTk",  [128, ESH], BF, kind="ExternalInput")      # Wl.T[:, e-shard]
    Wl2T  = nc.dram_tensor("Wl2T",  [128, 128], BF, kind="ExternalInput")      # Wl2.T
    WVT   = nc.dram_tensor("WVT",   [128, 128, ESH], BF, kind="ExternalInput") # WloT[e0+j, d] as [p, d, j]
    blk   = nc.dram_tensor("blk",   [128, ESH], FP, kind="ExternalInput")      # bl[e-shard] bcast
    bl2B  = nc.dram_tensor("bl2B",  [128, 128], FP, kind="ExternalInput")      # bl2 bcast
    bloB  = nc.dram_tensor("bloB",  [128, 128], FP, kind="ExternalInput")      # blo bcast
    xrow  = nc.dram_tensor("xrow",  [CB, D], FP, kind="ExternalInput")         # x rows of own c-shard
    gamB  = nc.dram_tensor("gamB",  [CB, D], FP, kind="ExternalInput")
    betB  = nc.dram_tensor("betB",  [CB, D], FP, kind="ExternalInput")
    out   = nc.dram_tensor("out",   [CB, D], FP, kind="ExternalOutput")

    Sqrt = mybir.ActivationFunctionType.Sqrt
    mult = mybir.AluOpType.mult

    with tile.TileContext(nc) as tc:
        with (
            tc.tile_pool(name="singles", bufs=1) as singles,
            tc.tile_pool(name="dram", bufs=1, space="DRAM") as dram,
            tc.tile_pool(name="gpool", bufs=2) as gpool,
            tc.tile_pool(name="h1pool", bufs=2) as h1pool,
            tc.tile_pool(name="h2pool", bufs=2) as h2pool,
            tc.tile_pool(name="h3pool", bufs=2) as h3pool,
            tc.tile_pool(name="ypool", bufs=2) as ypool,
            tc.tile_pool(name="pset", bufs=2, space="PSUM") as pset,
            tc.tile_pool(name="pmain", bufs=6, space="PSUM") as pmain,
        ):
            # ---- load inputs ----
            sb_xT = singles.tile([128, L], BF)
            nc.sync.dma_start(sb_xT, xT[:, :])
            sb_WlTk = singles.tile([128, ESH], BF)
            nc.sync.dma_start(sb_WlTk, WlTk[:, :])
            sb_Wl2T = singles.tile([128, 128], BF)
            nc.sync.dma_start(sb_Wl2T, Wl2T[:, :])
            sb_WVT = singles.tile([128, 128, ESH], BF)
            nc.sync.dma_start(sb_WVT, WVT[:, :, :])
            sb_blk = singles.tile([128, ESH], FP)
            nc.sync.dma_start(sb_blk, blk[:, :])
            sb_bl2B = singles.tile([128, 128], FP)
            nc.sync.dma_start(sb_bl2B, bl2B[:, :])
            sb_maskz = singles.tile([128, T, CB], BF)
            nc.sync.dma_start(sb_maskz, maskz[:, :, :])
            sb_maskT = singles.tile([128, T, L], BF)
            for t in range(T):
                nc.sync.dma_start(sb_maskT[:, t, :], maskT[:, t, :])
            sb_bloB = singles.tile([128, 128], FP)
            nc.scalar.dma_start(sb_bloB, bloB[:, :])
            sb_xrow = singles.tile([CB, D], FP)
            nc.scalar.dma_start(sb_xrow, xrow[:, :])
            sb_gamB = singles.tile([CB, D], FP)
            nc.scalar.dma_start(sb_gamB, gamB[:, :])
            sb_betB = singles.tile([CB, D], FP)
            nc.scalar.dma_start(sb_betB, betB[:, :])

            sb_eps = singles.tile([CB, 1], FP)
            nc.vector.memset(sb_eps, EPS)

            # ---- act_sel[a, j] (j in own e-shard) and tx[a, :] via PE ----
            act_sel = []
            tx_nat = []
            for t in range(T):
                ps = pset.tile([128, 144], FP, tag="set")
                xtile = sb_xT[:, t * 128:(t + 1) * 128]
                nc.tensor.matmul(ps[:, 0:ESH], xtile, sb_WlTk, start=True, stop=True)
                nc.tensor.matmul(ps[:, ESH:ESH + 128], xtile, sb_Wl2T, start=True, stop=True)
                a_t = singles.tile([128, ESH], BF, name=f"act_sel{t}")
                nc.vector.tensor_add(a_t, ps[:, 0:ESH], sb_blk)
                x_t = singles.tile([128, 128], BF, name=f"tx_nat{t}")
                nc.vector.tensor_add(x_t, ps[:, ESH:ESH + 128], sb_bl2B)
                act_sel.append(a_t)
                tx_nat.append(x_t)

            # ---- P2[t][a, d, j] = act[a,e_j] * tx[a,d]  (d-major; WloT folds
            # into the combine). One big tensor_tensor per a-tile.
            P2 = [singles.tile([128, 128, ESH], BF, name=f"P2_{t}") for t in range(T)]
            for t in range(T):
                nc.vector.tensor_mul(
                    P2[t],
                    tx_nat[t][:, :].unsqueeze(-1).broadcast_to((128, 128, ESH)),
                    act_sel[t][:, :].unsqueeze(1).broadcast_to((128, 128, ESH)),
                )

            # ---- main matmuls: S2[c, (d,j)] = sum_a mask[c,a] * P2[a, d, j] ----
            # q-pass structure: all 8 c-tiles for d-block q, then drain to SBUF.
            # Everything stays d-major so copies and combine are packed.
            S2T = [singles.tile([128, 128, ESH], BF, name=f"S2T{ct}") for ct in range(T)]
            for q in range(4):
                for ct in range(T):
                    ps = pmain.tile([128, 32, ESH], FP, tag="mm")
                    for t in range(T):
                        nc.tensor.matmul(
                            ps,
                            sb_maskT[:, t, ct * 128:(ct + 1) * 128],
                            P2[t][:, 32 * q:32 * q + 32, :],
                            start=(t == 0), stop=(t == T - 1),
                        )
                    nc.scalar.copy(S2T[ct][:, 32 * q:32 * q + 32, :], ps)

            # ---- Z matmul for own c-shard ----
            zps = pset.tile([128, 144], FP, tag="set")
            for t in range(T):
                nc.tensor.matmul(
                    zps[:, 0:128], sb_maskz[:, t, :], tx_nat[t],
                    start=(t == 0), stop=(t == T - 1),
                )
            sb_zb = singles.tile([CB, D], FP)
            nc.vector.tensor_mul(sb_zb, zps[:, 0:128], sb_bloB)

            # ---- combine: y1p[c,d] = sum_j act[c,e_j] * WloT[e_j,d] * S2[c,d,j] ----
            HF = mybir.dt.float16
            y1p_dram = dram.tile([L, D], HF)
            for ct in range(T):
                gw = gpool.tile([128, 128, ESH], BF, tag="gw")
                nc.vector.tensor_mul(gw, S2T[ct], sb_WVT)
                g = gpool.tile([128, 128, ESH], BF, tag="g")
                nc.vector.tensor_mul(
                    g, gw,
                    act_sel[ct][:, :].unsqueeze(1).broadcast_to((128, 128, ESH)),
                )
                h1 = h1pool.tile([128, 128, 8], BF, tag="h1")
                nc.vector.tensor_add(h1, g[:, :, 0:8], g[:, :, 8:16])
                h2 = h2pool.tile([128, 128, 4], BF, tag="h2")
                nc.vector.tensor_add(h2, h1[:, :, 0:4], h1[:, :, 4:8])
                h3 = h3pool.tile([128, 128, 2], BF, tag="h3")
                nc.vector.tensor_add(h3, h2[:, :, 0:2], h2[:, :, 2:4])
                y1 = ypool.tile([128, 128], HF, tag="y1")
                nc.vector.tensor_add(y1, h3[:, :, 0], h3[:, :, 1])
                nc.sync.dma_start(y1p_dram[ct * 128:(ct + 1) * 128, :], y1)

            # ---- ReduceScatter: core k gets sum over cores of rows [128k, 128k+128) ----
            rs_dram = dram.tile([CB, D], HF)
            nc.gpsimd.collective_compute(
                "ReduceScatter",
                mybir.AluOpType.add,
                replica_groups=[list(range(NCORES))],
                ins=[y1p_dram.opt()],
                outs=[rs_dram.opt()],
            )
            sb_rs = singles.tile([CB, D], HF)
            nc.sync.dma_start(sb_rs, rs_dram[:, :])

            # ---- residual + Z + LayerNorm ----
            y_sb = singles.tile([CB, D], FP)
            nc.vector.tensor_add(y_sb, sb_rs, sb_xrow)
            nc.vector.tensor_add(y_sb, y_sb, sb_zb)

            stats = singles.tile([CB, nc.vector.BN_STATS_DIM], FP)
            nc.vector.bn_stats(stats, y_sb)
            mv = singles.tile([CB, 2], FP)
            nc.vector.bn_aggr(mv, stats)
            nc.vector.tensor_scalar_sub(y_sb, y_sb, mv[:, 0:1])
            sd = singles.tile([CB, 1], FP)
            nc.scalar.activation(sd, mv[:, 1:2], Sqrt, bias=sb_eps, scale=1.0)
            rstd = singles.tile([CB, 1], FP)
            nc.vector.reciprocal(rstd, sd)
            nc.vector.tensor_scalar_mul(y_sb, y_sb, rstd)
            nc.vector.tensor_mul(y_sb, y_sb, sb_gamB)
            nc.vector.tensor_add(y_sb, y_sb, sb_betB)

            nc.sync.dma_start(out[:, :], y_sb)

    return nc


_NC_CACHE = None


def _get_nc():
    global _NC_CACHE
    if _NC_CACHE is None:
        _NC_CACHE = build_nc()
        _NC_CACHE.finalize()
    return _NC_CACHE


def _prepare_in_maps(x, mask, Wl, bl, Wlo, blo, Wl2, bl2, gamma, beta):
    f32 = np.float32
    x0 = np.asarray(x, f32)[0]                       # [L, D]
    m = np.asarray(mask)[0].astype(f32)              # [L, L]  (c, a)
    xT_bf = np.ascontiguousarray(x0.T).astype(BF_NP)
    WlT = np.asarray(Wl, f32).T                      # [d, e]
    Wl2T_bf = np.ascontiguousarray(np.asarray(Wl2, f32).T).astype(BF_NP)
    WloT = np.asarray(Wlo, f32).T                    # [e, d]
    mT = m.T.reshape(T, 128, L).transpose(1, 0, 2)   # [p, t, c]
    maskT_bf = np.ascontiguousarray(mT).astype(BF_NP)
    bl_ = np.asarray(bl, f32)
    bl2B = np.ascontiguousarray(np.broadcast_to(np.asarray(bl2, f32), (128, 128)))
    bloB = np.ascontiguousarray(np.broadcast_to(np.asarray(blo, f32), (128, 128)))
    gamB = np.ascontiguousarray(np.broadcast_to(np.asarray(gamma, f32), (CB, D)))
    betB = np.ascontiguousarray(np.broadcast_to(np.asarray(beta, f32), (CB, D)))

    in_maps = []
    for k in range(NCORES):
        esl = slice(k * ESH, (k + 1) * ESH)
        blkc = slice(k * CB, (k + 1) * CB)
        mz = m[blkc, :].T.reshape(T, 128, CB).transpose(1, 0, 2)  # [p, t, c']
        in_maps.append({
            "xT": xT_bf,
            "maskT": maskT_bf,
            "maskz": np.ascontiguousarray(mz).astype(BF_NP),
            "WlTk": np.ascontiguousarray(WlT[:, esl]).astype(BF_NP),
            "Wl2T": Wl2T_bf,
            "WVT": np.ascontiguousarray(
                np.broadcast_to(WloT[esl, :].T[None, :, :], (128, 128, ESH))
            ).astype(BF_NP),
            "blk": np.ascontiguousarray(
                np.broadcast_to(bl_[esl], (128, ESH))
            ),
            "bl2B": bl2B,
            "bloB": bloB,
            "xrow": np.ascontiguousarray(x0[blkc]),
            "gamB": gamB,
            "betB": betB,
        })
    return in_maps


def kernel(x, mask, Wl, bl, Wlo, blo, Wl2, bl2, gamma, beta):
    in_maps = _prepare_in_maps(x, mask, Wl, bl, Wlo, blo, Wl2, bl2, gamma, beta)
    res = run_bass_kernel_spmd(_get_nc(), in_maps, core_ids=list(range(NCORES)))
    y = np.concatenate([res.results[k]["out"] for k in range(NCORES)], axis=0)
    return y.reshape(B, L, D).astype(np.float32)


# revision 15
# speedup vs baseline: 1.9104x; 1.0519x over previous
"""Trainium2 Bass kernel for nn_JResCOPAttn (B=1, L=1024, D=128).

Reference computation:
    act = x @ Wl.T + bl                               # [L, E]  (E = D = 128)
    tm  = (act[:,None,:] * act[None,:,:]) @ Wlo.T + blo   # [L, L, D] (never materialized)
    tm *= (mask != 0)
    tx  = x @ Wl2.T + bl2                             # [L, D]
    y   = x + einsum('cad,ad->cd', tm, tx)
    out = LayerNorm(y) * gamma + beta

Algebraic restructuring (per output row c, channel d):
    y1[c,d] = sum_e act[c,e] * WloT[e,d] * S[c,e,d]  +  blo[d] * Z[c,d]
    S[c,e,d] = sum_a mask[c,a] * act[a,e] * tx[a,d]
    Z[c,d]   = sum_a mask[c,a] * tx[a,d]

Sharding: the e-dimension (128) is split across the 8 cores (16 e's each).
Each core computes P2[a,e,d] = act[a,e]*tx[a,d]*WloT[e,d] for its e-shard,
then S2 = maskT.T @ P2 as one large bf16 matmul (contraction over a=1024,
N=512 streams -> full PE rate; fp32 matmuls are 4x slower on TRN2).
The per-core partials y1p[c,d] = sum_{e in shard} act[c,e]*S2[c,e,d] are
summed with a ReduceScatter so core k ends up owning rows [128k, 128k+128),
where it adds the Z-term + residual and applies LayerNorm.
"""

import os
import sys

for _p in ("/opt/trn_rl_repo", "/root/.axon_site/_ro/trn_rl_repo"):
    if os.path.isdir(_p) and _p not in sys.path:
        sys.path.insert(0, _p)

import numpy as np
import ml_dtypes

import concourse.bass as bass
import concourse.tile as tile
from concourse import bacc, mybir
from concourse.bass_utils import run_bass_kernel_spmd

B, L, D = 1, 1024, 128
NCORES = 8
ESH = 16                  # e-channels per core
T = L // 128              # a-tiles = 8
CB = L // NCORES          # c-rows owned per core after ReduceScatter = 128
EPS = 1e-5
FP = mybir.dt.float32
BF = mybir.dt.bfloat16
BF_NP = ml_dtypes.bfloat16

N_DVE_J = 12              # P2-build: j < N_DVE_J on DVE, rest on gpsimd


def build_nc():
    nc = bacc.Bacc("TRN2", target_bir_lowering=False, num_devices=NCORES)

    # ---- I/O (per-core) ----
    xT    = nc.dram_tensor("xT",    [128, L], BF, kind="ExternalInput")        # x^T (d-major)
    maskT = nc.dram_tensor("maskT", [128, T, L], BF, kind="ExternalInput")     # [p,t,c] = mask[c, 128t+p]
    maskz = nc.dram_tensor("maskz", [128, T, CB], BF, kind="ExternalInput")    # own-shard columns
    WlTk  = nc.dram_tensor("WlTk",  [128, ESH], BF, kind="ExternalInput")      # Wl.T[:, e-shard]
    Wl2T  = nc.dram_tensor("Wl2T",  [128, 128], BF, kind="ExternalInput")      # Wl2.T
    WVT   = nc.dram_tensor("WVT",   [128, 128, ESH], BF, kind="ExternalInput") # WloT[e0+j, d] as [p, d, j]
    blk   = nc.dram_tensor("blk",   [128, ESH], FP, kind="ExternalInput")      # bl[e-shard] bcast
    bl2B  = nc.dram_tensor("bl2B",  [128, 128], FP, kind="ExternalInput")      # bl2 bcast
    bloB  = nc.dram_tensor("bloB",  [128, 128], FP, kind="ExternalInput")      # blo bcast
    xrow  = nc.dram_tensor("xrow",  [CB, D], FP, kind="ExternalInput")         # x rows of own c-shard
    gamB  = nc.dram_tensor("gamB",  [CB, D], FP, kind="ExternalInput")
    betB  = nc.dram_tensor("betB",  [CB, D], FP, kind="ExternalInput")
    out   = nc.dram_tensor("out",   [CB, D], FP, kind="ExternalOutput")

    Sqrt = mybir.ActivationFunctionType.Sqrt
    mult = mybir.AluOpType.mult

    with tile.TileContext(nc) as tc:
        with (
            tc.tile_pool(name="singles", bufs=1) as singles,
            tc.tile_pool(name="dram", bufs=1, space="DRAM") as dram,
            tc.tile_pool(name="gpool", bufs=2) as gpool,
            tc.tile_pool(name="h1pool", bufs=2) as h1pool,
            tc.tile_pool(name="h2pool", bufs=2) as h2pool,
            tc.tile_pool(name="h3pool", bufs=2) as h3pool,
            tc.tile_pool(name="ypool", bufs=2) as ypool,
            tc.tile_pool(name="pmain", bufs=2, space="PSUM") as pmain,
        ):
            # ---- load inputs ----
            sb_xT = singles.tile([128, L], BF)
            nc.sync.dma_start(sb_xT, xT[:, :])
            sb_WlTk = singles.tile([128, ESH], BF)
            nc.sync.dma_start(sb_WlTk, WlTk[:, :])
            sb_Wl2T = singles.tile([128, 128], BF)
            nc.sync.dma_start(sb_Wl2T, Wl2T[:, :])
            sb_WVT = singles.tile([128, 128, ESH], BF)
            nc.sync.dma_start(sb_WVT, WVT[:, :, :])
            sb_blk = singles.tile([128, ESH], FP)
            nc.sync.dma_start(sb_blk, blk[:, :])
            sb_bl2B = singles.tile([128, 128], FP)
            nc.sync.dma_start(sb_bl2B, bl2B[:, :])
            sb_maskz = singles.tile([128, T, CB], BF)
            nc.sync.dma_start(sb_maskz, maskz[:, :, :])
            sb_maskT = singles.tile([128, T, L], BF)
            for t in range(T):
                nc.sync.dma_start(sb_maskT[:, t, :], maskT[:, t, :])
            sb_bloB = singles.tile([128, 128], FP)
            nc.scalar.dma_start(sb_bloB, bloB[:, :])
            sb_xrow = singles.tile([CB, D], FP)
            nc.scalar.dma_start(sb_xrow, xrow[:, :])
            sb_gamB = singles.tile([CB, D], FP)
            nc.scalar.dma_start(sb_gamB, gamB[:, :])
            sb_betB = singles.tile([CB, D], FP)
            nc.scalar.dma_start(sb_betB, betB[:, :])

            sb_eps = singles.tile([CB, 1], FP)
            nc.vector.memset(sb_eps, EPS)

            # ---- act_sel[a, j] (j in own e-shard) and tx[a, :] via PE ----
            act_sel = []
            tx_nat = []
            for t in range(T):
                ps = pmain.tile([128, 128, ESH], FP, tag="mm4")
                xtile = sb_xT[:, t * 128:(t + 1) * 128]
                nc.tensor.matmul(ps[:, 0, 0:ESH], xtile, sb_WlTk, start=True, stop=True)
                nc.tensor.matmul(ps[:, 1:9, :], xtile, sb_Wl2T, start=True, stop=True)
                a_t = singles.tile([128, ESH], BF, name=f"act_sel{t}")
                nc.vector.tensor_add(a_t, ps[:, 0, 0:ESH], sb_blk)
                x_t = singles.tile([128, 128], BF, name=f"tx_nat{t}")
                nc.vector.tensor_add(x_t, ps[:, 1:9, :], sb_bl2B)
                act_sel.append(a_t)
                tx_nat.append(x_t)

            # ---- Z matmul for own c-shard (fills the P2-gated PE bubble) ----
            zps = pmain.tile([128, 128, ESH], FP, tag="mm4")
            for t in range(T):
                nc.tensor.matmul(
                    zps[:, 0:8, :], sb_maskz[:, t, :], tx_nat[t],
                    start=(t == 0), stop=(t == T - 1),
                )
            sb_zb = singles.tile([CB, D], FP)
            nc.vector.tensor_mul(sb_zb, zps[:, 0:8, :], sb_bloB)

            # ---- P2[t][a, d, j] = act[a,e_j] * tx[a,d]  (d-major; WloT folds
            # into the combine). One big tensor_tensor per a-tile.
            P2 = [singles.tile([128, 128, ESH], BF, name=f"P2_{t}") for t in range(T)]
            for t in range(T):
                nc.vector.tensor_mul(
                    P2[t],
                    tx_nat[t][:, :].unsqueeze(-1).broadcast_to((128, 128, ESH)),
                    act_sel[t][:, :].unsqueeze(1).broadcast_to((128, 128, ESH)),
                )

            # ---- per c-tile: matmuls, combine, y1p DMA ----
            # S2[c, (d,j)] = sum_a mask[c,a] * P2[a, d, j]; a-tile order rotated
            # per ct so early c-tiles can start before all P2 tiles are built.
            y1p_dram = dram.tile([L, D], FP)
            for ct in range(T):
                ps4 = pmain.tile([128, 128, ESH], FP, tag="mm4")
                for i in range(T):
                    t = (ct + i) % T
                    for q in range(4):
                        nc.tensor.matmul(
                            ps4[:, 32 * q:32 * q + 32, :],
                            sb_maskT[:, t, ct * 128:(ct + 1) * 128],
                            P2[t][:, 32 * q:32 * q + 32, :],
                            start=(i == 0), stop=(i == T - 1),
                            skip_group_check=True,
                        )
                # drain PSUM -> SBUF (packed) on the scalar engine
                s2 = gpool.tile([128, 128, ESH], BF, tag="s2")
                for q in range(4):
                    nc.scalar.copy(
                        s2[:, 32 * q:32 * q + 32, :], ps4[:, 32 * q:32 * q + 32, :]
                    )
                # combine: y1p[c,d] = sum_j act[c,e_j] * WloT[e_j,d] * S2[c,d,j]
                gw = gpool.tile([128, 128, ESH], BF, tag="gw")
                nc.vector.tensor_mul(gw, s2, sb_WVT)
                g = gpool.tile([128, 128, ESH], BF, tag="g")
                nc.vector.tensor_mul(
                    g, gw,
                    act_sel[ct][:, :].unsqueeze(1).broadcast_to((128, 128, ESH)),
                )
                h1 = h1pool.tile([128, 128, 8], BF, tag="h1")
                nc.vector.tensor_add(h1, g[:, :, 0:8], g[:, :, 8:16])
                h2 = h2pool.tile([128, 128, 4], BF, tag="h2")
                nc.vector.tensor_add(h2, h1[:, :, 0:4], h1[:, :, 4:8])
                h3 = h3pool.tile([128, 128, 2], BF, tag="h3")
                nc.vector.tensor_add(h3, h2[:, :, 0:2], h2[:, :, 2:4])
                y1 = ypool.tile([128, 128], FP, tag="y1")
                nc.vector.tensor_add(y1, h3[:, :, 0], h3[:, :, 1])
                nc.sync.dma_start(y1p_dram[ct * 128:(ct + 1) * 128, :], y1)

            # ---- ReduceScatter: core k gets sum over cores of rows [128k, 128k+128) ----
            rs_dram = dram.tile([CB, D], FP)
            nc.gpsimd.collective_compute(
                "ReduceScatter",
                mybir.AluOpType.add,
                replica_groups=[list(range(NCORES))],
                ins=[y1p_dram.opt()],
                outs=[rs_dram.opt()],
            )
            sb_rs = singles.tile([CB, D], FP)
            nc.sync.dma_start(sb_rs, rs_dram[:, :])

            # ---- residual + Z + LayerNorm ----
            y_sb = singles.tile([CB, D], FP)
            nc.vector.tensor_add(y_sb, sb_rs, sb_xrow)
            nc.vector.tensor_add(y_sb, y_sb, sb_zb)

            stats = singles.tile([CB, nc.vector.BN_STATS_DIM], FP)
            nc.vector.bn_stats(stats, y_sb)
            mv = singles.tile([CB, 2], FP)
            nc.vector.bn_aggr(mv, stats)
            nc.vector.tensor_scalar_sub(y_sb, y_sb, mv[:, 0:1])
            sd = singles.tile([CB, 1], FP)
            nc.scalar.activation(sd, mv[:, 1:2], Sqrt, bias=sb_eps, scale=1.0)
            rstd = singles.tile([CB, 1], FP)
            nc.vector.reciprocal(rstd, sd)
            nc.vector.tensor_scalar_mul(y_sb, y_sb, rstd)
            nc.vector.tensor_mul(y_sb, y_sb, sb_gamB)
            nc.vector.tensor_add(y_sb, y_sb, sb_betB)

            nc.sync.dma_start(out[:, :], y_sb)

    return nc


_NC_CACHE = None


def _get_nc():
    global _NC_CACHE
    if _NC_CACHE is None:
        _NC_CACHE = build_nc()
        _NC_CACHE.finalize()
    return _NC_CACHE


def _prepare_in_maps(x, mask, Wl, bl, Wlo, blo, Wl2, bl2, gamma, beta):
    f32 = np.float32
    x0 = np.asarray(x, f32)[0]                       # [L, D]
    m = np.asarray(mask)[0].astype(f32)              # [L, L]  (c, a)
    xT_bf = np.ascontiguousarray(x0.T).astype(BF_NP)
    WlT = np.asarray(Wl, f32).T                      # [d, e]
    Wl2T_bf = np.ascontiguousarray(np.asarray(Wl2, f32).T).astype(BF_NP)
    WloT = np.asarray(Wlo, f32).T                    # [e, d]
    mT = m.T.reshape(T, 128, L).transpose(1, 0, 2)   # [p, t, c]
    maskT_bf = np.ascontiguousarray(mT).astype(BF_NP)
    bl_ = np.asarray(bl, f32)
    bl2B = np.ascontiguousarray(np.broadcast_to(np.asarray(bl2, f32), (128, 128)))
    bloB = np.ascontiguousarray(np.broadcast_to(np.asarray(blo, f32), (128, 128)))
    gamB = np.ascontiguousarray(np.broadcast_to(np.asarray(gamma, f32), (CB, D)))
    betB = np.ascontiguousarray(np.broadcast_to(np.asarray(beta, f32), (CB, D)))

    in_maps = []
    for k in range(NCORES):
        esl = slice(k * ESH, (k + 1) * ESH)
        blkc = slice(k * CB, (k + 1) * CB)
        mz = m[blkc, :].T.reshape(T, 128, CB).transpose(1, 0, 2)  # [p, t, c']
        in_maps.append({
            "xT": xT_bf,
            "maskT": maskT_bf,
            "maskz": np.ascontiguousarray(mz).astype(BF_NP),
            "WlTk": np.ascontiguousarray(WlT[:, esl]).astype(BF_NP),
            "Wl2T": Wl2T_bf,
            "WVT": np.ascontiguousarray(
                np.broadcast_to(WloT[esl, :].T[None, :, :], (128, 128, ESH))
            ).astype(BF_NP),
            "blk": np.ascontiguousarray(
                np.broadcast_to(bl_[esl], (128, ESH))
            ),
            "bl2B": bl2B,
            "bloB": bloB,
            "xrow": np.ascontiguousarray(x0[blkc]),
            "gamB": gamB,
            "betB": betB,
        })
    return in_maps


def kernel(x, mask, Wl, bl, Wlo, blo, Wl2, bl2, gamma, beta):
    in_maps = _prepare_in_maps(x, mask, Wl, bl, Wlo, blo, Wl2, bl2, gamma, beta)
    res = run_bass_kernel_spmd(_get_nc(), in_maps, core_ids=list(range(NCORES)))
    y = np.concatenate([res.results[k]["out"] for k in range(NCORES)], axis=0)
    return y.reshape(B, L, D).astype(np.float32)
